# revision 3
# baseline (speedup 1.0000x reference)
"""GAT (2-layer, 2-head) + MLP head on 8 TRN2 cores — v3.

Instruction-count-minimal redesign (HW dispatches ~60-110us per instruction
regardless of size):
  - Edge aggregation via conflict-free dma_scatter_add into f32 DRAM rows.
    Edges bucketed by (dst, rank//4); each bucket's <=4 edges sit at one
    partition in 4 consecutive chunks and are pre-summed by one DVE reduce;
    buckets of one dst have distinct rounds, and scatter windows never span
    rounds, so within-instruction dst indices are unique (in-instruction
    duplicate RMW races avoided; cross-instruction accumulation exact —
    verified on HW).
  - Attention logits packed into the gatherable row tensors; per-edge
    expansion via 1024-idx dma_gathers (src + dst side).
  - Node-feature transposes via XBAR dma transpose (1 instr/128-col strip).
  - GEMMs in channel-major orientation minimizing matmul count.
  - softmax without max-subtraction (same math; logits are O(1)).
Host does only graph-index prep + weight reshaping; all x-dependent compute
runs on device.
"""
import sys

sys.path.insert(0, "/opt/trn_rl_repo")

import numpy as np
import ml_dtypes

import concourse.bass as bass
import concourse.bacc as bacc
import concourse.mybir as mybir
import concourse.tile as tile
from concourse.bass import broadcast_tensor_aps
from concourse.bass_utils import run_bass_kernel_spmd

F32 = mybir.dt.float32
BF16 = mybir.dt.bfloat16
F8 = mybir.dt.float8e4
PM2 = mybir.MatmulPerfMode.DoubleRow
I16 = mybir.dt.int16
AF = mybir.ActivationFunctionType
ALU = mybir.AluOpType

N = 10000
E = 160000
OUT = 3
NCORES = 8
NLOC = N // NCORES            # 1250
NPAD = 1280                   # local rows padded to 10 blocks of 128
BK = 4                        # edges pre-summed per bucket
XAW = 384                     # xa row units: [x 256 | a1 4*f32=8u | pad]
HRW = 640                     # hrow units: [h1 512 | a2 4*f32=8u | pad]
ANW = 128                     # anode2 row units: [a2 8u | pad]
Z1S = 576                     # Z1 row stride f32 (2304B)
Z2S = 1088                    # Z2 row stride f32 (4352B)
ZROWS = NPAD + 1024           # Z rows: 1280 real + 1024 pad-scratch

bf = lambda a: np.asarray(a, ml_dtypes.bfloat16)


def _wrap_tab(idx):
    epad = len(idx)
    t = idx.astype(np.int16).reshape(epad // 16, 16).T
    return np.ascontiguousarray(np.tile(t, (8, 1)))


# ----------------------------------------------------------------- host prep

def _prep(inputs):
    """Graph-structure prep (data-independent).

    Global round-aligned bucket layout (identical shape on every core):
    round r holds the r-th bucket (rank//BK == r) of each dst, padded to the
    max count over cores and to a 128 multiple; total padded to 1024-mult.
    Bucket position p -> presum tile [p%128, p//128]; its BK edges at gather
    slots ((p//128)*BK + boff)*128 + p%128. Scatter windows are 1024-aligned
    cuts within rounds -> unique dst rows per window on every core.
    """
    ei = np.asarray(inputs["edge_index"])
    src_g = np.concatenate([ei[0], np.arange(N)]).astype(np.int64)
    dst_g = np.concatenate([ei[1], np.arange(N)]).astype(np.int64)
    own = dst_g // NLOC

    pc = []
    nrounds = 0
    for c in range(NCORES):
        sel = own == c
        s = src_g[sel]
        dl = dst_g[sel] - c * NLOC
        o = np.argsort(dl, kind="stable")
        s, dl = s[o], dl[o]
        starts = np.r_[0, np.flatnonzero(np.diff(dl)) + 1]
        runlen = np.diff(np.r_[starts, len(dl)])
        rank = np.arange(len(dl)) - np.repeat(starts, runlen)
        bidx = rank // BK
        boff = rank % BK
        nrounds = max(nrounds, int(bidx.max()) + 1)
        pc.append((s, dl, bidx, boff))

    rsz = np.zeros(nrounds, np.int64)
    for c in range(NCORES):
        s, dl, bidx, boff = pc[c]
        for r in range(nrounds):
            rsz[r] = max(rsz[r], len(np.unique(dl[bidx == r])))
    rsz = ((rsz + 127) // 128) * 128
    rof = np.concatenate([[0], np.cumsum(rsz)])
    nbuck = int(rof[-1])
    NBUCK = ((nbuck + 1023) // 1024) * 1024
    NSLOT = NBUCK * BK
    NCH = NSLOT // 128
    NGRP = NBUCK // 1024
    NGI = NSLOT // 1024

    wins = []
    for r in range(nrounds):
        a, b = int(rof[r]), int(rof[r + 1])
        while a < b:
            e_ = min(b, (a // 1024 + 1) * 1024)
            wins.append((a, e_ - a))
            a = e_

    stabs, dtabs, sctabs, pmasks = [], [], [], []
    for c in range(NCORES):
        s, dl, bidx, boff = pc[c]
        bpos = np.full(len(s), -1, np.int64)
        bdl = np.full(NBUCK, -1, np.int64)
        for r in range(nrounds):
            m = bidx == r
            dr = dl[m]
            u, inv = np.unique(dr, return_inverse=True)
            bpos[m] = rof[r] + inv
            bdl[rof[r]:rof[r] + len(u)] = u
        slot = ((bpos // 128) * BK + boff) * 128 + (bpos % 128)
        gsrc = np.zeros(NSLOT, np.int64)
        gdst = np.full(NSLOT, N, np.int64)     # pad -> poison row (a=-1e30)
        gsrc[slot] = s
        gdst[slot] = dl + c * NLOC
        srow = NPAD + (np.arange(NBUCK) % 1024)
        real = bdl >= 0
        srow[real] = bdl[real]
        stabs.append(_wrap_tab(gsrc))
        dtabs.append(_wrap_tab(gdst))
        sctabs.append(_wrap_tab(srow))

    return dict(NBUCK=NBUCK, NSLOT=NSLOT, NCH=NCH, NGRP=NGRP, NGI=NGI,
                nbuck=nbuck, wins=wins, stabs=stabs, dtabs=dtabs,
                sctabs=sctabs)


def _make_in_maps(inputs, P):
    x = np.asarray(inputs["x"], np.float32)
    w1s = np.asarray(inputs["w1s"], np.float32)        # [256, 512]
    w1d = np.asarray(inputs["w1d"], np.float32)
    w2s = np.asarray(inputs["w2s"], np.float32)        # [512, 1024]
    w2d = np.asarray(inputs["w2d"], np.float32)

    def fold(W, att):
        h, cc = att.shape
        return np.stack([W[:, i * cc:(i + 1) * cc] @ att[i] for i in range(h)], 1)

    v1 = np.concatenate([fold(w1s, np.asarray(inputs["a1s"], np.float32)),
                         fold(w1d, np.asarray(inputs["a1d"], np.float32))], 1)
    v2 = np.concatenate([fold(w2s, np.asarray(inputs["a2s"], np.float32)),
                         fold(w2d, np.asarray(inputs["a2d"], np.float32))], 1)

    xa = np.zeros((N + 16, XAW), ml_dtypes.bfloat16)
    xa[0:N, 0:256] = bf(x)

    w1sT = w1s.reshape(2, 128, 2, 2, 128).transpose(1, 0, 2, 3, 4)
    w1sT = np.ascontiguousarray(w1sT.reshape(128, 2, 4, 128))
    w2sT = w2s.reshape(4, 128, 2, 4, 128).transpose(1, 0, 2, 3, 4)
    w2sT = np.ascontiguousarray(w2sT.reshape(128, 4, 8, 128))
    wf1 = np.asarray(inputs["wf1"], np.float32)
    wf1T = np.ascontiguousarray(wf1.reshape(8, 128, 128).transpose(1, 0, 2))
    b1 = np.asarray(inputs["b1"], np.float32)
    b2 = np.asarray(inputs["b2"], np.float32)
    common = {
        "v1a": bf(np.ascontiguousarray(v1.reshape(2, 128, 4).transpose(1, 0, 2))),
        "v2a": bf(np.ascontiguousarray(v2.reshape(4, 128, 4).transpose(1, 0, 2))),
        "w1sT": bf(w1sT),
        "w2sT": bf(w2sT),
        "wf1T": bf(wf1T),
        "wf2": bf(np.asarray(inputs["wf2"], np.float32)),
        "b1T": np.ascontiguousarray(b1.reshape(4, 128).T),
        "b2T": np.ascontiguousarray(b2.reshape(8, 128).T),
        "bf1c": np.asarray(inputs["bf1"], np.float32)[:, None],
        "bf2c": np.asarray(inputs["bf2"], np.float32)[:, None],
        "gb": np.stack([np.asarray(inputs["gamma"], np.float32),
                        np.asarray(inputs["beta"], np.float32)], 1),
        "ones3": bf(np.ones((3, 1), np.float32)),
        "ones13": bf(np.ones((1, 3), np.float32)),
    }
    xT = np.ascontiguousarray(x.T.reshape(2, 128, N).transpose(1, 0, 2))
    maps = []
    for c in range(NCORES):
        m = dict(common)
        m["xa"] = xa
        xtl = np.zeros((128, 2, NPAD), np.float32)
        xtl[:, :, 0:NLOC] = xT[:, :, c * NLOC:(c + 1) * NLOC]
        m["xTloc"] = bf(xtl)
        m["stab"] = P["stabs"][c]
        m["dtab"] = P["dtabs"][c]
        m["sctab"] = P["sctabs"][c]
        maps.append(m)
    return maps


# ------------------------------------------------------------- kernel build

def build(P, repeat=1):
    NCH, NGRP = P["NCH"], P["NGRP"]
    NBUCK = P["NBUCK"]
    wins = P["wins"]
    CPG = NCH // NGRP             # chunks per group (32)
    BPG = NBUCK // NGRP           # buckets per group (1024)

    nc = bacc.Bacc("TRN2", target_bir_lowering=False, debug=False,
                   num_devices=NCORES, num_swdge_queues=2)

    def din(name, shape, dt):
        return nc.dram_tensor(name, shape, dt, kind="ExternalInput")

    xa_d = din("xa", [N + 16, XAW], BF16)
    xTloc_d = din("xTloc", [128, 2, NPAD], BF16)
    v1a_d = din("v1a", [128, 2, 4], BF16)
    v2a_d = din("v2a", [128, 4, 4], BF16)
    w1sT_d = din("w1sT", [128, 2, 4, 128], BF16)
    w2sT_d = din("w2sT", [128, 4, 8, 128], BF16)
    wf1T_d = din("wf1T", [128, 8, 128], BF16)
    wf2_d = din("wf2", [128, OUT], BF16)
    b1T_d = din("b1T", [128, 4], F32)
    b2T_d = din("b2T", [128, 8], F32)
    bf1c_d = din("bf1c", [128, 1], F32)
    bf2c_d = din("bf2c", [OUT, 1], F32)
    gb_d = din("gb", [128, 2], F32)
    ones3_d = din("ones3", [OUT, 1], BF16)
    ones13_d = din("ones13", [1, OUT], BF16)
    stab_d = din("stab", [128, P["NSLOT"] // 16], I16)
    dtab_d = din("dtab", [128, P["NSLOT"] // 16], I16)
    sctab_d = din("sctab", [128, NBUCK // 16], I16)
    out_d = nc.dram_tensor("out", [NLOC, OUT], F32, kind="ExternalOutput")

    def body(tc):
        with tc.tile_pool(name="const", bufs=1) as cp, \
             tc.tile_pool(name="dram", bufs=1, space="DRAM") as dram:
            def load(name, shape, dt, src):
                t = cp.tile(shape, dt, tag=name)
                nc.sync.dma_start(out=t[:], in_=src[:])
                return t

            v1a = load("v1a", [128, 2, 4], BF16, v1a_d)
            v2a = load("v2a", [128, 4, 4], BF16, v2a_d)
            w1sT = load("w1sT", [128, 2, 4, 128], BF16, w1sT_d)
            w2sT = load("w2sT", [128, 4, 8, 128], BF16, w2sT_d)
            wf1T = load("wf1T", [128, 8, 128], BF16, wf1T_d)
            wf2 = load("wf2", [128, OUT], BF16, wf2_d)
            b1T = load("b1T", [128, 4], F32, b1T_d)
            b2T = load("b2T", [128, 8], F32, b2T_d)
            bf1c = load("bf1c", [128, 1], F32, bf1c_d)
            bf2c = load("bf2c", [OUT, 1], F32, bf2c_d)
            gb = load("gb", [128, 2], F32, gb_d)
            ones3 = load("ones3", [OUT, 1], BF16, ones3_d)
            ones13 = load("ones13", [1, OUT], BF16, ones13_d)
            stab = load("stab", [128, P["NSLOT"] // 16], I16, stab_d)
            dtab = load("dtab", [128, P["NSLOT"] // 16], I16, dtab_d)
            sctab = load("sctab", [128, NBUCK // 16], I16, sctab_d)

            _regs = {}

            def reg(v):
                if v not in _regs:
                    _regs[v] = nc.gpsimd.to_reg(v)
                return _regs[v]

            reg(1024)
            for (_w0, _wn) in P["wins"]:
                reg(_wn)

            xaw = dram.tile([N + 16, XAW], BF16)
            nc.sync.dma_start(out=xaw[:], in_=xa_d[:])
            Z1 = dram.tile([ZROWS, Z1S], F32)
            Z2 = dram.tile([ZROWS, Z2S], F32)
            Znr1 = dram.tile([NPAD, 512], BF16)
            Znr2 = dram.tile([NPAD, 1024], BF16)
            h1Tt = dram.tile([512, NPAD], BF16)
            hrow_my = dram.tile([NLOC, HRW], BF16)
            hrow = dram.tile([N, HRW], BF16)
            anode1my = dram.tile([NLOC, ANW], BF16)
            anode1 = dram.tile([N + 16, ANW], BF16)
            anode2 = dram.tile([N + 16, ANW], BF16)
            poison = cp.tile([1, 4], F32, tag="poison")
            nc.vector.memset(poison[:], -1e30)
            an1pf = anode1[:].bitcast(F32)
            nc.sync.dma_start(out=an1pf[N:N + 1, 0:4], in_=poison[:])
            an2pf = anode2[:].bitcast(F32)
            nc.sync.dma_start(out=an2pf[N:N + 1, 0:4], in_=poison[:])
            statmy = dram.tile([128, 2], F32)
            statall = dram.tile([128 * NCORES, 2], F32)

            def phase_a():
                """local a1 logits -> AllGather -> xa rows (words 128:132)."""
                with tc.tile_pool(name="pa", bufs=1) as pa, \
                     tc.tile_pool(name="pap", bufs=1, space="PSUM") as pap:
                    xT = pa.tile([128, 2, NPAD], BF16)
                    nc.sync.dma_start(out=xT[:], in_=xTloc_d[:])
                    ps = pap.tile([4, NPAD], F32, tag="psA")
                    for c0 in range(0, NPAD, 512):
                        w = min(512, NPAD - c0)
                        for kc in range(2):
                            nc.tensor.matmul(ps[:, c0:c0 + w], v1a[:, kc, :],
                                             xT[:, kc, c0:c0 + w],
                                             start=(kc == 0), stop=(kc == 1))
                    a1sb = pa.tile([4, NPAD], F32)
                    nc.scalar.activation(a1sb[:], ps[:], AF.Copy)
                    an1f = anode1my[:].bitcast(F32)
                    nc.sync.dma_start(
                        out=an1f[0:NLOC, 0:4].rearrange("n p -> p n"),
                        in_=a1sb[:, 0:NLOC])
                nc.gpsimd.collective_compute(
                    "AllGather", ALU.bypass,
                    replica_groups=[list(range(NCORES))],
                    ins=[anode1my.opt()], outs=[anode1[0:N, :].opt()])
                xawf = xaw[:].bitcast(F32)
                an1ff = anode1[:].bitcast(F32)
                nc.sync.dma_start(out=xawf[0:N, 128:132],
                                  in_=an1ff[0:N, 0:4])

            def edge_phase(lay, rowt, runits, su, dstt, dunits, doff,
                           Z, MSG, ZS, gxbufs, gdbufs):
                """Gather rows, compute p, scale, presum, scatter into Z."""
                ZE = 2 * MSG + 2
                real_nch = P["nbuck"] // 32   # chunks with any real slot
                with tc.tile_pool(name=f"ez{lay}", bufs=1) as zp:
                    zz = zp.tile([128, 10, ZS], F32)
                    nc.vector.memset(zz[:], 0.0)
                    nc.sync.dma_start(
                        out=Z[0:NPAD, :].rearrange("(b p) e -> p b e", p=128),
                        in_=zz[:])
                with tc.tile_pool(name=f"eg{lay}", bufs=gxbufs) as gp, \
                     tc.tile_pool(name=f"ed{lay}", bufs=gdbufs) as dp, \
                     tc.tile_pool(name=f"ew{lay}", bufs=1) as wp, \
                     tc.tile_pool(name=f"ep{lay}", bufs=2 if lay == 1 else 1) as ppre:
                    for g in range(NGRP):
                        ch0 = g * CPG
                        gx = gp.tile([128, CPG, runits], BF16, tag="gx")
                        gd = dp.tile([128, CPG, dunits], BF16, tag="gd")
                        pad_q = None
                        for q in range(CPG // 8):
                            if ch0 + q * 8 >= real_nch:
                                pad_q = q
                                break
                            i0 = (ch0 + q * 8) * 8
                            nc.gpsimd.dma_gather(
                                gx[:, q * 8:(q + 1) * 8, :],
                                rowt[:, 0:runits],
                                stab[:, i0:i0 + 64],
                                num_idxs=1024, num_idxs_reg=reg(1024),
                                elem_size=runits, elem_step=runits,
                                queue_num=0)
                            nc.gpsimd.dma_gather(
                                gd[:, q * 8:(q + 1) * 8, :],
                                dstt[:, 0:dunits],
                                dtab[:, i0:i0 + 64],
                                num_idxs=1024, num_idxs_reg=reg(1024),
                                elem_size=dunits, elem_step=dunits,
                                queue_num=1)
                        if pad_q is not None:
                            nc.vector.memset(gx[:, pad_q * 8:, :], 0.0)
                            nc.vector.memset(gd[:, pad_q * 8:, :], -1e30)
                        gxf = gx[:].bitcast(F32)
                        gdf = gd[:].bitcast(F32)
                        ee = wp.tile([128, CPG, 2], F32, tag="ee")
                        nc.vector.tensor_add(ee[:], gxf[:, :, su:su + 2],
                                             gdf[:, :, doff:doff + 2])
                        lk = wp.tile([128, CPG, 2], F32, tag="lk")
                        nc.vector.scalar_tensor_tensor(
                            lk[:], ee[:], 0.2, ee[:], ALU.mult, ALU.max)
                        srows = wp.tile([128, CPG, ZE], BF16, tag="srows")
                        nc.scalar.activation(srows[:, :, 2 * MSG:ZE], lk[:],
                                             AF.Exp)
                        for h in range(2):
                            a_, b_ = broadcast_tensor_aps(
                                gx[:, :, 0:MSG],
                                srows[:, :, 2 * MSG + h:2 * MSG + h + 1])
                            nc.vector.tensor_tensor(
                                srows[:, :, h * MSG:(h + 1) * MSG],
                                a_, b_, ALU.mult)
                        pre = ppre.tile([128, CPG // BK, ZE], F32, tag="pre")
                        nc.vector.tensor_reduce(
                            pre[:],
                            srows[:].rearrange("p (c k) e -> p c e k", k=BK),
                            mybir.AxisListType.X, ALU.add)
                        for (w0, wn) in wins:
                            if not (g * BPG <= w0 < (g + 1) * BPG):
                                continue
                            o0 = (w0 - g * BPG) // 128
                            nc.gpsimd.dma_scatter_add(
                                Z[:, 0:ZE], pre[:, o0:o0 + wn // 128, :],
                                sctab[:, w0 // 16:(w0 + wn) // 16],
                                num_idxs=wn, num_idxs_reg=reg(wn),
                                elem_size=ZE, elem_step=ZS, queue_num=0)

            def l1_post():
                with tc.tile_pool(name="p1", bufs=1) as p1, \
                     tc.tile_pool(name="p1p", bufs=1, space="PSUM") as p1p:
                    zld = p1.tile([128, 10, 514], F32)
                    nc.sync.dma_start(
                        out=zld[:],
                        in_=Z1[0:NPAD, 0:514].rearrange("(b p) e -> p b e", p=128))
                    rec = p1.tile([128, 10, 2], F32)
                    nc.vector.reciprocal(rec[:], zld[:, :, 512:514])
                    znb = p1.tile([128, 10, 512], BF16)
                    for h in range(2):
                        a_, b_ = broadcast_tensor_aps(
                            zld[:, :, h * 256:(h + 1) * 256],
                            rec[:, :, h:h + 1])
                        nc.vector.tensor_tensor(
                            znb[:, :, h * 256:(h + 1) * 256], a_, b_, ALU.mult)
                    nc.sync.dma_start(
                        out=Znr1[:].rearrange("(b p) e -> p b e", p=128),
                        in_=znb[:])
                    znT = p1.tile([128, 4, NPAD], BF16)
                    for k in range(4):
                        nc.sync.dma_start(out=znT[:, k, :],
                                          in_=Znr1[:, k * 128:(k + 1) * 128],
                                          transpose=True)
                    h1T = p1.tile([128, 4, NPAD], BF16)
                    for h in range(2):
                        for o in range(2):
                            ps = p1p.tile([128, NPAD], F32, tag="ps1")
                            for kc in range(2):
                                for c0 in range(0, NPAD, 512):
                                    w = min(512, NPAD - c0)
                                    nc.tensor.matmul(
                                        ps[:, c0:c0 + w],
                                        w1sT[:, kc, h * 2 + o, :],
                                        znT[:, h * 2 + kc, c0:c0 + w],
                                        start=(kc == 0), stop=(kc == 1))
                            nc.scalar.activation(
                                h1T[:, h * 2 + o, :], ps[:], AF.Relu,
                                bias=b1T[:, h * 2 + o:h * 2 + o + 1])
                    ps4 = p1p.tile([4, NPAD], F32, tag="ps4")
                    for kc in range(4):
                        for c0 in range(0, NPAD, 512):
                            w = min(512, NPAD - c0)
                            nc.tensor.matmul(ps4[:, c0:c0 + w], v2a[:, kc, :],
                                             h1T[:, kc, c0:c0 + w],
                                             start=(kc == 0), stop=(kc == 3))
                    a2sb = p1.tile([4, NPAD], F32)
                    nc.scalar.activation(a2sb[:], ps4[:], AF.Copy)
                    nc.sync.dma_start(
                        out=h1Tt[:].rearrange("(k p) n -> p k n", p=128),
                        in_=h1T[:])
                    h1r = p1.tile([128, 10, 512], BF16)
                    for b in range(10):
                        nc.sync.dma_start(out=h1r[:, b, :],
                                          in_=h1Tt[:, b * 128:(b + 1) * 128],
                                          transpose=True)
                    nc.sync.dma_start(
                        out=hrow_my[0:1152, 0:512].rearrange(
                            "(b p) e -> p b e", p=128),
                        in_=h1r[:, 0:9, :])
                    nc.sync.dma_start(out=hrow_my[1152:NLOC, 0:512],
                                      in_=h1r[0:98, 9, :])
                    hrf = hrow_my[:].bitcast(F32)
                    nc.sync.dma_start(
                        out=hrf[0:NLOC, 256:260].rearrange("n p -> p n"),
                        in_=a2sb[:, 0:NLOC])
                nc.gpsimd.collective_compute(
                    "AllGather", ALU.bypass,
                    replica_groups=[list(range(NCORES))],
                    ins=[hrow_my.opt()], outs=[hrow.opt()])
                nc.sync.dma_start(out=anode2[0:N, 0:8], in_=hrow[:, 512:520])

            def l2_post_and_head():
                with tc.tile_pool(name="p2", bufs=1) as p2:
                    with tc.tile_pool(name="p2p", bufs=1, space="PSUM") as p2p:
                        zld = p2.tile([128, 10, 1026], F32)
                        nc.sync.dma_start(
                            out=zld[:],
                            in_=Z2[0:NPAD, 0:1026].rearrange(
                                "(b p) e -> p b e", p=128))
                        rec = p2.tile([128, 10, 2], F32)
                        nc.vector.reciprocal(rec[:], zld[:, :, 1024:1026])
                        znb = p2.tile([128, 10, 1024], BF16)
                        for h in range(2):
                            a_, b_ = broadcast_tensor_aps(
                                zld[:, :, h * 512:(h + 1) * 512],
                                rec[:, :, h:h + 1])
                            nc.vector.tensor_tensor(
                                znb[:, :, h * 512:(h + 1) * 512], a_, b_,
                                ALU.mult)
                        nc.sync.dma_start(
                            out=Znr2[:].rearrange("(b p) e -> p b e", p=128),
                            in_=znb[:])
                        znT = p2.tile([128, 8, NPAD], BF16)
                        for k in range(8):
                            nc.sync.dma_start(out=znT[:, k, :],
                                              in_=Znr2[:, k * 128:(k + 1) * 128],
                                              transpose=True)
                        h2T = p2.tile([128, 8, NPAD], BF16)
                        for h in range(2):
                            for o in range(4):
                                ps = p2p.tile([128, NPAD], F32, tag="ps2")
                                for kc in range(4):
                                    for c0 in range(0, NPAD, 512):
                                        w = min(512, NPAD - c0)
                                        nc.tensor.matmul(
                                            ps[:, c0:c0 + w],
                                            w2sT[:, kc, h * 4 + o, :],
                                            znT[:, h * 4 + kc, c0:c0 + w],
                                            start=(kc == 0), stop=(kc == 3))
                                nc.scalar.activation(
                                    h2T[:, h * 4 + o, :], ps[:], AF.Relu,
                                    bias=b2T[:, h * 4 + o:h * 4 + o + 1])
                        ps = p2p.tile([128, NPAD], F32, tag="ps2")
                        for kc in range(8):
                            for c0 in range(0, NPAD, 512):
                                w = min(512, NPAD - c0)
                                nc.tensor.matmul(ps[:, c0:c0 + w],
                                                 wf1T[:, kc, :],
                                                 h2T[:, kc, c0:c0 + w],
                                                 start=(kc == 0), stop=(kc == 7))
                        tTs = p2.tile([128, NPAD], F32)
                        nc.vector.tensor_scalar_add(tTs[:], ps[:], bf1c[:, 0:1])
                    nc.vector.memset(tTs[:, NLOC:NPAD], 0.0)
                    sq = p2.tile([128, NPAD], F32)
                    statp = p2.tile([128, 2], F32)
                    nc.scalar.activation(sq[:], tTs[:], AF.Square,
                                         accum_out=statp[:, 1:2])
                    nc.vector.tensor_reduce(statp[:, 0:1], tTs[:],
                                            mybir.AxisListType.X, ALU.add)
                    nc.sync.dma_start(out=statmy[:], in_=statp[:])
                    nc.gpsimd.collective_compute(
                        "AllGather", ALU.bypass,
                        replica_groups=[list(range(NCORES))],
                        ins=[statmy.opt()], outs=[statall.opt()])
                    st8 = p2.tile([128, 8, 2], F32)
                    nc.sync.dma_start(
                        out=st8[:],
                        in_=statall[:].rearrange("(c p) e -> p c e", p=128))
                    st = p2.tile([128, 2], F32)
                    nc.vector.tensor_reduce(
                        st[:], st8[:].rearrange("p c e -> p e c"),
                        mybir.AxisListType.X, ALU.add)
                    mean = p2.tile([128, 1], F32)
                    nc.vector.tensor_scalar_mul(mean[:], st[:, 0:1], 1.0 / N)
                    msq = p2.tile([128, 1], F32)
                    nc.vector.tensor_scalar_mul(msq[:], st[:, 1:2], 1.0 / N)
                    var = p2.tile([128, 1], F32)
                    nc.vector.tensor_mul(var[:], mean[:], mean[:])
                    nc.vector.tensor_sub(var[:], msq[:], var[:])
                    nc.vector.tensor_scalar_add(var[:], var[:], 1e-5)
                    vr = p2.tile([128, 1], F32)
                    nc.vector.reciprocal(vr[:], var[:])
                    rstd = p2.tile([128, 1], F32)
                    nc.scalar.activation(rstd[:], vr[:], AF.Sqrt)
                    scl = p2.tile([128, 1], F32)
                    nc.vector.tensor_mul(scl[:], rstd[:], gb[:, 0:1])
                    sh = p2.tile([128, 1], F32)
                    nc.vector.tensor_mul(sh[:], mean[:], scl[:])
                    nc.vector.tensor_sub(sh[:], gb[:, 1:2], sh[:])
                    tn = p2.tile([128, NPAD], BF16)
                    nc.scalar.activation(tn[:], tTs[:], AF.Relu,
                                         scale=scl[:, 0:1], bias=sh[:, 0:1])
                    with tc.tile_pool(name="php", bufs=1, space="PSUM") as php:
                        ps3 = php.tile([OUT, NPAD], F32, tag="ps3")
                        for c0 in range(0, NPAD, 512):
                            w = min(512, NPAD - c0)
                            nc.tensor.matmul(ps3[:, c0:c0 + w], wf2[:],
                                             tn[:, c0:c0 + w],
                                             start=True, stop=True)
                        po = p2.tile([OUT, NPAD], F32)
                        nc.vector.tensor_scalar_add(po[:], ps3[:], bf2c[:, 0:1])
                        ex = p2.tile([OUT, NPAD], BF16)
                        nc.scalar.activation(ex[:], po[:], AF.Exp)
                        ps1 = php.tile([1, NPAD], F32, tag="ps1h")
                        for c0 in range(0, NPAD, 512):
                            w = min(512, NPAD - c0)
                            nc.tensor.matmul(ps1[:, c0:c0 + w], ones3[:],
                                             ex[:, c0:c0 + w],
                                             start=True, stop=True)
                        ln = p2.tile([1, NPAD], BF16)
                        nc.scalar.activation(ln[:], ps1[:], AF.Ln)
                        ps3b = php.tile([OUT, NPAD], F32, tag="ps3")
                        for c0 in range(0, NPAD, 512):
                            w = min(512, NPAD - c0)
                            nc.tensor.matmul(ps3b[:, c0:c0 + w], ones13[:],
                                             ln[:, c0:c0 + w],
                                             start=True, stop=True)
                        outv = p2.tile([OUT, NPAD], F32)
                        nc.vector.tensor_sub(outv[:], po[:], ps3b[:])
                        nc.sync.dma_start(
                            out=out_d[:, :].rearrange("n p -> p n"),
                            in_=outv[:, 0:NLOC])

            for _rep in range(repeat):
                phase_a()
                edge_phase(1, xaw, XAW, 128, anode1, ANW, 2, Z1, 256, Z1S,
                           2, 2)
                l1_post()
                edge_phase(2, hrow, HRW, 256, anode2, ANW, 2, Z2, 512, Z2S,
                           1, 2)
                l2_post_and_head()

    with tile.TileContext(nc) as tc:
        body(tc)
    nc.compile()
    return nc


# ------------------------------------------------------------------ runner

def kernel(**inputs):
    P = _prep(inputs)
    in_maps = _make_in_maps(inputs, P)
    nc = build(P)
    res = run_bass_kernel_spmd(nc, in_maps, list(range(NCORES)))
    out = np.concatenate([res.results[c]["out"] for c in range(NCORES)], 0)
    return out.astype(np.float32)


if __name__ == "__main__":
    rng = np.random.default_rng(0)
    fake = {"edge_index": rng.integers(0, N, (2, E))}
    P = _prep(fake)
    print("NBUCK", P["NBUCK"], "NSLOT", P["NSLOT"], "NCH", P["NCH"],
          "NGRP", P["NGRP"], "NGI", P["NGI"], "wins", len(P["wins"]))
    print("win sizes:", [w[1] for w in P["wins"]])
